# revision 46
# baseline (speedup 1.0000x reference)
"""BalancedL1Loss Trainium2 kernel (8 NeuronCores, pure data parallel).

The loss only needs 33 global scalars:
  - C_b   = #elements with t >= edge_b            (16 count "tail" sums)
  - T_b   = sum |o - t| over elements t >= edge_b (16 weighted "tail" sums)
  - S_tot = sum |o - t| over ALL elements
Per-bin histogram counts / L1-sums are adjacent differences of the tails;
the EMA + weight + final ratio is O(16) host math done in float64.  The
t >= edge_b compares use the exact f32 bin edges, bit-identical to the
reference's searchsorted(side='right') semantics.

Sharding: batch dim 64 -> 8 batches per core; each core's shard is laid
out as [128 partitions, 16384] f32 and processed in 4 chunks.

Device pipeline per chunk (the shipped "v3" builder; v1/v2/v4 variants are
kept for benchmarking):
  VectorE : diff = o - t (f32->bf16); for the first _DVE_MASK_EDGES edges a
            fused tensor_scalar(is_ge)+accum produces the 0/1 mask (bf16)
            AND the exact count tail; for every edge a bf16 tensor_tensor
            multiplies mask * |diff|.
  ScalarE : |diff| via Abs; for the remaining edges Sign(t - e) + accum
            produces a +-1 mask and a sign-sum (host decodes
            C = (sign_sum + N) / 2, T = (signed_tail + S_tot) / 2).
  TensorE : ones-vector matmuls accumulate column sums of each product
            into one PSUM row-segment per edge (quadrant rows 0/32/64),
            plus an S_tot row from |diff| itself; one final tensor_reduce
            collapses PSUM [128, 8x512] -> [128, 8].
Per-chunk count partials and the tail block are DMA'd out per core and
combined on host in float64.

Measured on trn2 (slope-timed over repeat-66 vs repeat-2 NEFFs to cancel
the ~80 ms axon tunnel dispatch overhead): ~285 us per full pass across
8 cores, vs ~47 us memory roofline and ~607 us for the naive all-DVE
version.  The kernel is compute-bound: 17 independent bin functionals
need 17 mask/sign passes on the elementwise engines (one output function
per pass), split across VectorE and ScalarE.
"""

import numpy as np

_NCORES = 8
_P = 128
_FULL_BATCH = 64
_B_PER_CORE = _FULL_BATCH // _NCORES  # 8
_ELEM_PER_CORE = _B_PER_CORE * 512 * 512  # 2097152
_FD = _ELEM_PER_CORE // _P  # 16384
_NCHUNK = 4
_NBIN = 16
_NCOL = 2 * _NBIN + 1  # 16 count tails + 16 weighted tails + 1 total
_EDGES = np.arange(0.2, 1.0, 0.05).astype(np.float32)  # exact reference bins

_MOMENTUM = 0.9
_GAMMA = 0.5
_REPEAT_THR = 1.0
_LOSS_WEIGHT = 1.0

LAST_EXEC_NS = None
TRACE = False

_compiled_cache = {}


def _build(fd=_FD, nchunk=_NCHUNK, debug=False, repeat=1, counts="act_sign"):
    """Emit the Bass program for one core: inputs o,t [128, fd] f32,
    output acc [128, nchunk*_NCOL] f32 of per-partition partial sums.

    counts="dve_ts":   C_b tails via DVE tensor_scalar(is_ge)+accum.
    counts="act_sign": sign-sums via ScalarE Sign activation + accum
                       (host decodes C_b = (sum_sign + numel) / 2), freeing
                       the vector engine for the 17 weighted-tail passes.
    repeat>1 re-runs the whole pass (for slope-based HW timing)."""
    import concourse.bacc as bacc
    import concourse.mybir as mybir
    from concourse.tile import TileContext

    assert fd % nchunk == 0
    cw = fd // nchunk
    f32 = mybir.dt.float32
    bf16 = mybir.dt.bfloat16
    op = mybir.AluOpType
    act_fn = mybir.ActivationFunctionType

    nc = bacc.Bacc("TRN2", target_bir_lowering=False, debug=debug)
    o_d = nc.dram_tensor("o", [_P, fd], f32, kind="ExternalInput")
    t_d = nc.dram_tensor("t", [_P, fd], f32, kind="ExternalInput")
    acc_d = nc.dram_tensor("acc", [_P, nchunk * _NCOL], f32, kind="ExternalOutput")

    with TileContext(nc) as tc:
        with (
            tc.tile_pool(name="io", bufs=2) as io,
            tc.tile_pool(name="accp", bufs=1) as accp,
        ):
            # Separate accumulator tiles per engine so ScalarE and VectorE
            # accum writes never serialize on a shared tile.
            acc_v = accp.tile([_P, nchunk * (_NBIN + 1)], f32)
            acc_s = accp.tile([_P, nchunk * _NBIN], f32)
            zbias = accp.tile([_P, 1], f32)
            nc.vector.memset(zbias[:], 0.0)
            ebias = accp.tile([_P, _NBIN], f32)
            for b in range(_NBIN):
                nc.vector.memset(ebias[:, b : b + 1], -float(_EDGES[b]))
            for c in [c for _ in range(repeat) for c in range(nchunk)]:
                o_t = io.tile([_P, cw], f32, tag="o")
                t_t = io.tile([_P, cw], f32, tag="t")
                l1 = io.tile([_P, cw], f32, tag="l1")
                scr = io.tile([_P, cw], f32, tag="scr")
                nc.sync.dma_start(o_t[:], o_d[:, c * cw : (c + 1) * cw])
                nc.sync.dma_start(t_t[:], t_d[:, c * cw : (c + 1) * cw])
                nc.vector.tensor_tensor(
                    out=scr[:], in0=o_t[:], in1=t_t[:], op=op.subtract
                )
                # |diff| on the scalar engine (abs_max is not a legal DVE
                # tensor_scalar/tensor_tensor op on CoreV3).
                nc.scalar.activation(
                    out=l1[:], in_=scr[:], func=act_fn.Abs, bias=zbias[:]
                )
                if counts == "act_sign":
                    scr_s = io.tile([_P, cw], bf16, tag="scr_s")
                    for b in range(_NBIN):
                        nc.scalar.activation(
                            out=scr_s[:],
                            in_=t_t[:],
                            func=act_fn.Sign,
                            bias=ebias[:, b : b + 1],
                            accum_out=acc_s[:, c * _NBIN + b : c * _NBIN + b + 1],
                        )
                else:
                    for b in range(_NBIN):
                        nc.vector.tensor_scalar(
                            out=scr[:],
                            in0=t_t[:],
                            scalar1=float(_EDGES[b]),
                            scalar2=None,
                            op0=op.is_ge,
                            op1=op.add,
                            accum_out=acc_s[:, c * _NBIN + b : c * _NBIN + b + 1],
                        )
                # 17th "edge" of -1.0 is always true: gives S_tot = sum |o-t|.
                base = c * (_NBIN + 1)
                for b in range(_NBIN + 1):
                    e = float(_EDGES[b]) if b < _NBIN else -1.0
                    nc.vector.scalar_tensor_tensor(
                        out=scr[:],
                        in0=t_t[:],
                        scalar=e,
                        in1=l1[:],
                        op0=op.is_ge,
                        op1=op.mult,
                        accum_out=acc_v[:, base + b : base + b + 1],
                    )
            nc.sync.dma_start(acc_d[:, : nchunk * (_NBIN + 1)], acc_v[:])
            nc.sync.dma_start(acc_d[:, nchunk * (_NBIN + 1) :], acc_s[:])
    nc.compile()
    nc._counts_mode = counts
    return nc


def _build_v3(
    fd=_FD,
    nchunk=_NCHUNK,
    debug=False,
    repeat=1,
    dve_mask_edges=4,
):
    """v3: per edge, build a mask once (DVE tensor_scalar+accum for the first
    `dve_mask_edges` edges -> exact count tails; ScalarE Sign+accum for the
    rest -> sign sums), multiply by |o-t| in bf16 on DVE, and reduce the
    products with TensorE ones-matmuls accumulating into one PSUM row per
    edge.  Row 16 accumulates |o-t| itself (S_tot).  A final tiny reduce
    collapses PSUM [17, 512] -> [17, 1].

    acc layout: cols 0..nchunk*16-1 = per-chunk count partials
    (exact counts for DVE-mask edges, sign-sums for ACT edges);
    col nchunk*16 = tails in rows 0..16 (T_b for DVE edges, 2*T_b - S_tot
    for ACT edges, S_tot in row 16)."""
    import concourse.bacc as bacc
    import concourse.mybir as mybir
    from concourse.tile import TileContext

    assert fd % nchunk == 0
    cw = fd // nchunk
    nslab = (cw + 511) // 512
    assert cw % 512 == 0
    f32 = mybir.dt.float32
    bf16 = mybir.dt.bfloat16
    op = mybir.AluOpType
    act_fn = mybir.ActivationFunctionType
    NB = _NBIN

    nc = bacc.Bacc("TRN2", target_bir_lowering=False, debug=debug)
    o_d = nc.dram_tensor("o", [_P, fd], f32, kind="ExternalInput")
    t_d = nc.dram_tensor("t", [_P, fd], f32, kind="ExternalInput")
    ncol = nchunk * NB + 8
    acc_d = nc.dram_tensor("acc", [_P, ncol], f32, kind="ExternalOutput")

    with TileContext(nc) as tc:
        with (
            tc.tile_pool(name="io", bufs=2) as io,
            tc.tile_pool(name="accp", bufs=1) as accp,
            tc.tile_pool(name="psum", bufs=1, space="PSUM") as psp,
        ):
            acc_c = accp.tile([_P, nchunk * NB], f32)
            acc_t = accp.tile([_P, 8], f32)
            ones = accp.tile([_P, 1], bf16)
            nc.vector.memset(ones[:], 1.0)
            zbias = accp.tile([_P, 1], f32)
            nc.vector.memset(zbias[:], 0.0)
            ebias = accp.tile([_P, NB], f32)
            for b in range(NB):
                nc.vector.memset(ebias[:, b : b + 1], -float(_EDGES[b]))
            # One PSUM row-segment per edge: tails for edge b accumulate at
            # psum partition 32*(b//8), columns [512*(b%8), 512*(b%8+1));
            # S_tot at partition 64, columns 0..511.  PE output rows can only
            # land on quadrant partitions {0,32,64,96}, hence the layout.
            ptail = psp.tile([_P, 4096], f32)
            nc.vector.memset(ptail[:], 0.0)

            def row_seg(b):
                if b == NB:
                    return 64, 0
                return 32 * (b // 8), b % 8

            first = [True] * (NB + 1)
            for ci, c in enumerate(
                [c for _ in range(repeat) for c in range(nchunk)]
            ):
                o_t = io.tile([_P, cw], f32, tag="o")
                t_t = io.tile([_P, cw], f32, tag="t")
                diff = io.tile([_P, cw], bf16, tag="diff")
                l1 = io.tile([_P, cw], bf16, tag="l1")
                mask = io.tile([_P, cw], bf16, tag="mask")
                prod = io.tile([_P, cw], bf16, tag="prod")
                nc.sync.dma_start(o_t[:], o_d[:, c * cw : (c + 1) * cw])
                nc.sync.dma_start(t_t[:], t_d[:, c * cw : (c + 1) * cw])
                nc.vector.tensor_tensor(
                    out=diff[:], in0=o_t[:], in1=t_t[:], op=op.subtract
                )
                nc.scalar.activation(
                    out=l1[:], in_=diff[:], func=act_fn.Abs, bias=zbias[:]
                )
                # S_tot row: accumulate column sums of l1
                q, seg = row_seg(NB)
                for s in range(nslab):
                    nc.tensor.matmul(
                        ptail[q : q + 1, seg * 512 : (seg + 1) * 512],
                        ones[:],
                        l1[:, s * 512 : (s + 1) * 512],
                        start=first[NB],
                        stop=(ci == repeat * nchunk - 1 and s == nslab - 1),
                        tile_position=(0, q),
                    )
                    first[NB] = False
                for b in range(NB):
                    col = c * NB + b
                    if b < dve_mask_edges:
                        nc.vector.tensor_scalar(
                            out=mask[:],
                            in0=t_t[:],
                            scalar1=float(_EDGES[b]),
                            scalar2=None,
                            op0=op.is_ge,
                            op1=op.add,
                            accum_out=acc_c[:, col : col + 1],
                        )
                    else:
                        nc.scalar.activation(
                            out=mask[:],
                            in_=t_t[:],
                            func=act_fn.Sign,
                            bias=ebias[:, b : b + 1],
                            accum_out=acc_c[:, col : col + 1],
                        )
                    nc.vector.tensor_tensor(
                        out=prod[:], in0=mask[:], in1=l1[:], op=op.mult
                    )
                    q, seg = row_seg(b)
                    for s in range(nslab):
                        nc.tensor.matmul(
                            ptail[q : q + 1, seg * 512 : (seg + 1) * 512],
                            ones[:],
                            prod[:, s * 512 : (s + 1) * 512],
                            start=first[b],
                            stop=(ci == repeat * nchunk - 1 and s == nslab - 1),
                            tile_position=(0, q),
                        )
                        first[b] = False
            nc.vector.tensor_reduce(
                out=acc_t[:],
                in_=ptail[:].rearrange("p (g s) -> p g s", g=8),
                axis=mybir.AxisListType.X,
                op=op.add,
            )
            nc.sync.dma_start(acc_d[:, : nchunk * NB], acc_c[:])
            nc.sync.dma_start(acc_d[:, nchunk * NB :], acc_t[:])
    nc.compile()
    return nc


def _build_v4(
    fd=_FD,
    nchunk=_NCHUNK,
    debug=False,
    repeat=1,
    dve_mask_edges=9,
    wave=4,
):
    """v4: like v3 but the 16 per-edge product+reduce DVE passes are replaced
    by TensorE column-dot matmuls: for each 128-col slab,
    psum_block_b[m, n] += sum_p l1[p, slab_m] * mask_b[p, slab_n]; the
    DIAGONAL of block b accumulates the per-column-group weighted tails.
    A final identity-weighted scalar_tensor_tensor per edge extracts the
    diagonal into per-partition partials summed on host.

    acc layout: cols 0..nchunk*16-1 = per-chunk count partials (exact counts
    for DVE-mask edges, sign-sums for ACT edges); cols nchunk*16 .. +17 =
    per-partition diag partials (T for DVE edges, 2T - S_tot for ACT edges,
    S_tot last)."""
    import concourse.bacc as bacc
    import concourse.mybir as mybir
    from concourse.tile import TileContext

    assert fd % nchunk == 0
    cw = fd // nchunk
    assert cw % 128 == 0
    nslab = cw // 128
    f32 = mybir.dt.float32
    bf16 = mybir.dt.bfloat16
    op = mybir.AluOpType
    act_fn = mybir.ActivationFunctionType
    NB = _NBIN

    nc = bacc.Bacc("TRN2", target_bir_lowering=False, debug=debug)
    o_d = nc.dram_tensor("o", [_P, fd], f32, kind="ExternalInput")
    t_d = nc.dram_tensor("t", [_P, fd], f32, kind="ExternalInput")
    id_d = nc.dram_tensor("ident", [_P, _P], f32, kind="ExternalInput")
    ncol = nchunk * NB + NB + 1
    acc_d = nc.dram_tensor("acc", [_P, ncol], f32, kind="ExternalOutput")

    waves = [list(range(w, min(w + wave, NB))) for w in range(0, NB, wave)]

    with TileContext(nc) as tc:
        with (
            tc.tile_pool(name="io", bufs=2) as io,
            tc.tile_pool(name="mk", bufs=2) as mk,
            tc.tile_pool(name="accp", bufs=1) as accp,
            tc.tile_pool(name="psum", bufs=1, space="PSUM") as psp,
        ):
            acc_c = accp.tile([_P, nchunk * NB], f32)
            acc_t = accp.tile([_P, NB + 1], f32)
            ones128 = accp.tile([_P, _P], bf16)
            nc.vector.memset(ones128[:], 1.0)
            ident = accp.tile([_P, _P], f32)
            nc.sync.dma_start(ident[:], id_d[:])
            zbias = accp.tile([_P, 1], f32)
            nc.vector.memset(zbias[:], 0.0)
            ebias = accp.tile([_P, NB], f32)
            for b in range(NB):
                nc.vector.memset(ebias[:, b : b + 1], -float(_EDGES[b]))
            # 17 psum blocks of [128, 128] f32; block b's diagonal holds the
            # per-column-group tail sums for edge b (b=16: S_tot).  PSUM has
            # only 8 accumulation-group banks, so instead of start/stop
            # groups the region is zeroed once and every matmul accumulates
            # (start=False).
            ptail = psp.tile([_P, (NB + 1) * _P], f32)
            nc.vector.memset(ptail[:], 0.0)
            first = [False] * (NB + 1)
            last_ci = repeat * nchunk - 1
            for ci, c in enumerate(
                [c for _ in range(repeat) for c in range(nchunk)]
            ):
                o_t = io.tile([_P, cw], f32, tag="o")
                t_t = io.tile([_P, cw], f32, tag="t")
                diff = io.tile([_P, cw], bf16, tag="diff")
                l1 = io.tile([_P, cw], bf16, tag="l1")
                nc.sync.dma_start(o_t[:], o_d[:, c * cw : (c + 1) * cw])
                nc.sync.dma_start(t_t[:], t_d[:, c * cw : (c + 1) * cw])
                nc.vector.tensor_tensor(
                    out=diff[:], in0=o_t[:], in1=t_t[:], op=op.subtract
                )
                nc.scalar.activation(
                    out=l1[:], in_=diff[:], func=act_fn.Abs, bias=zbias[:]
                )
                # S_tot block: diag += column dots of l1 against ones
                for s in range(nslab):
                    nc.tensor.matmul(
                        ptail[:, NB * _P : (NB + 1) * _P],
                        l1[:, s * _P : (s + 1) * _P],
                        ones128[:],
                        start=False,
                        stop=(ci == last_ci and s == nslab - 1),
                        skip_group_check=True,
                    )
                for wv in waves:
                    masks = {}
                    for j, b in enumerate(wv):
                        m = mk.tile([_P, cw], bf16, tag=f"mask{j}")
                        masks[b] = m
                        col = c * NB + b
                        if b < dve_mask_edges:
                            nc.vector.tensor_scalar(
                                out=m[:],
                                in0=t_t[:],
                                scalar1=float(_EDGES[b]),
                                scalar2=None,
                                op0=op.is_ge,
                                op1=op.add,
                                accum_out=acc_c[:, col : col + 1],
                            )
                        else:
                            nc.scalar.activation(
                                out=m[:],
                                in_=t_t[:],
                                func=act_fn.Sign,
                                bias=ebias[:, b : b + 1],
                                accum_out=acc_c[:, col : col + 1],
                            )
                    for s in range(nslab):
                        for b in wv:
                            nc.tensor.matmul(
                                ptail[:, b * _P : (b + 1) * _P],
                                l1[:, s * _P : (s + 1) * _P],
                                masks[b][:, s * _P : (s + 1) * _P],
                                start=False,
                                stop=(ci == last_ci and s == nslab - 1),
                                skip_group_check=True,
                            )
            # Diagonal extraction: acc_t[p, b] = sum_n ptail_b[p, n]*ident[p, n]
            # = ptail_b[p, p]; host sums over partitions.
            scr_d = accp.tile([_P, _P], f32)
            for b in range(NB + 1):
                nc.vector.scalar_tensor_tensor(
                    out=scr_d[:],
                    in0=ptail[:, b * _P : (b + 1) * _P],
                    scalar=1.0,
                    in1=ident[:],
                    op0=op.mult,
                    op1=op.mult,
                    accum_out=acc_t[:, b : b + 1],
                )
            nc.sync.dma_start(acc_d[:, : nchunk * NB], acc_c[:])
            nc.sync.dma_start(acc_d[:, nchunk * NB :], acc_t[:])
    nc.compile()
    return nc


_COUNTS_MODE = "act_sign"
_VERSION = "v3"
_DVE_MASK_EDGES = 6


def _get_compiled(repeat=1):
    key = ("nc", repeat, _VERSION, _COUNTS_MODE, _DVE_MASK_EDGES)
    if key not in _compiled_cache:
        if _VERSION == "v4":
            _compiled_cache[key] = _build_v4(
                repeat=repeat, dve_mask_edges=_DVE_MASK_EDGES
            )
        elif _VERSION == "v3":
            _compiled_cache[key] = _build_v3(
                repeat=repeat, dve_mask_edges=_DVE_MASK_EDGES
            )
        else:
            _compiled_cache[key] = _build(repeat=repeat, counts=_COUNTS_MODE)
    return _compiled_cache[key]


def _finish(acc_partials, counts, numel, counts_mode="act_sign", nchunk=_NCHUNK):
    """acc_partials: float array [..., P, nchunk*17 + nchunk*16] of
    per-partition partials; reduces in f64 and applies the EMA/weight math."""
    flat = acc_partials.astype(np.float64).reshape(-1, acc_partials.shape[-1])
    nt = nchunk * (_NBIN + 1)
    tails = flat[:, :nt].reshape(-1, _NBIN + 1).sum(axis=0)
    csums = flat[:, nt:].reshape(-1, _NBIN).sum(axis=0)
    T = tails[:_NBIN]
    s_tot = tails[_NBIN]
    if counts_mode == "act_sign":
        # csums are sum(sign(t - e)) = (#t>e) - (#t<e); C = (csum + numel)/2
        C = (csums + float(numel)) / 2.0
    else:
        C = csums
    N = np.empty(_NBIN)
    S = np.empty(_NBIN)
    N[:-1] = C[:-1] - C[1:]
    N[-1] = C[-1]
    S[:-1] = T[:-1] - T[1:]
    S[-1] = T[-1]
    n_inv = numel - C[0]
    s_inv = s_tot - T[0]

    new_counts = _MOMENTUM * counts.astype(np.float64) + (1.0 - _MOMENTUM) * N
    freq = new_counts / new_counts.sum()
    wi = (_REPEAT_THR / freq) ** _GAMMA
    num = float((S * wi).sum() + s_inv)
    den = float((N * wi).sum() + n_inv)
    return np.float32(num / den * _LOSS_WEIGHT)


def _get_exec(repeat=1):
    """Build (once) the sharded jitted executable over 8 cores.

    Mirrors concourse.bass2jax.run_bass_via_pjrt's multi-core tail, but keeps
    the jitted function so repeated calls reuse the compiled NEFF and inputs
    can stay device-resident for benchmarking."""
    key = ("exec", repeat, _COUNTS_MODE)
    if key in _compiled_cache:
        return _compiled_cache[key]

    import jax
    import concourse.mybir as mybir
    from concourse import bass2jax
    from jax.experimental.shard_map import shard_map
    from jax.sharding import Mesh, PartitionSpec

    nc = _get_compiled(repeat=repeat)
    bass2jax.install_neuronx_cc_hook()

    partition_name = (
        nc.partition_id_tensor.name if nc.partition_id_tensor else None
    )
    in_names = []
    out_names = []
    out_avals = []
    zero_outs = []
    for alloc in nc.m.functions[0].allocations:
        if not isinstance(alloc, mybir.MemoryLocationSet):
            continue
        name = alloc.memorylocations[0].name
        if alloc.kind == "ExternalInput":
            if name != partition_name:
                in_names.append(name)
        elif alloc.kind == "ExternalOutput":
            out_names.append(name)
            shape = tuple(alloc.tensor_shape)
            dtype = mybir.dt.np(alloc.dtype)
            out_avals.append(jax.core.ShapedArray(shape, dtype))
            zero_outs.append(np.zeros(shape, dtype))
    n_params = len(in_names)
    n_outs = len(out_avals)
    all_names = list(in_names) + list(out_names)
    if partition_name is not None:
        all_names.append(partition_name)
    donate = tuple(range(n_params, n_params + n_outs))

    def _body(*args):
        operands = list(args)
        if partition_name is not None:
            operands.append(bass2jax.partition_id_tensor())
        outs = bass2jax._bass_exec_p.bind(
            *operands,
            out_avals=tuple(out_avals),
            in_names=tuple(all_names),
            out_names=tuple(out_names),
            lowering_input_output_aliases=(),
            sim_require_finite=True,
            sim_require_nnan=True,
            nc=nc,
        )
        return tuple(outs)

    devices = jax.devices()[:_NCORES]
    mesh = Mesh(np.asarray(devices), ("core",))
    in_specs = (PartitionSpec("core"),) * (n_params + n_outs)
    out_specs = (PartitionSpec("core"),) * n_outs
    sharded = jax.jit(
        shard_map(
            _body, mesh=mesh, in_specs=in_specs, out_specs=out_specs,
            check_rep=False,
        ),
        donate_argnums=donate,
        keep_unused=True,
    )
    info = {
        "fn": sharded,
        "mesh": mesh,
        "in_names": in_names,
        "out_names": out_names,
        "out_avals": out_avals,
        "zero_outs": zero_outs,
        "n_params": n_params,
    }
    _compiled_cache[key] = info
    return info


def _shard_inputs(outputs, targets):
    """Concatenated global inputs: [8*128, FD] with core i's shard at rows
    [128i, 128(i+1))."""
    o = outputs.reshape(_NCORES, _P, _FD).reshape(_NCORES * _P, _FD)
    t = targets.reshape(_NCORES, _P, _FD).reshape(_NCORES * _P, _FD)
    ins = {"o": np.ascontiguousarray(o), "t": np.ascontiguousarray(t)}
    if _VERSION == "v4":
        ident = np.eye(_P, dtype=np.float32)
        ins["ident"] = np.tile(ident, (_NCORES, 1))
    return ins


def _run_concat(concat_in):
    """concat_in: dict name -> global array. Returns acc [8, 128, NCHUNK*NCOL]."""
    info = _get_exec()
    args = [concat_in[name] for name in info["in_names"]]
    zeros = [
        np.zeros((_NCORES * z.shape[0], *z.shape[1:]), z.dtype)
        for z in info["zero_outs"]
    ]
    out_arrs = info["fn"](*args, *zeros)
    acc = np.asarray(out_arrs[info["out_names"].index("acc")])
    return acc.reshape(_NCORES, _P, -1)


def _finish_v3(acc, counts_in, numel, dve_mask_edges=None, nchunk=_NCHUNK):
    if dve_mask_edges is None:
        dve_mask_edges = _DVE_MASK_EDGES
    """acc: [..., P, nchunk*16 + 1] per-core partials from _build_v3."""
    a = acc.astype(np.float64)
    a = a.reshape(-1, a.shape[-2], a.shape[-1])  # [cores, P, ncol]
    csums = a[:, :, : nchunk * _NBIN].reshape(-1, _NBIN).sum(axis=0)
    tails8 = a[:, :, nchunk * _NBIN :].sum(axis=0)  # [P, 8]
    s_tot = tails8[64, 0]
    C = np.empty(_NBIN)
    T = np.empty(_NBIN)
    for b in range(_NBIN):
        t_b = tails8[32 * (b // 8), b % 8]
        if b < dve_mask_edges:
            C[b] = csums[b]
            T[b] = t_b
        else:
            C[b] = (csums[b] + float(numel)) / 2.0
            T[b] = (t_b + s_tot) / 2.0
    N = np.empty(_NBIN)
    S = np.empty(_NBIN)
    N[:-1] = C[:-1] - C[1:]
    N[-1] = C[-1]
    S[:-1] = T[:-1] - T[1:]
    S[-1] = T[-1]
    n_inv = numel - C[0]
    s_inv = s_tot - T[0]
    new_counts = _MOMENTUM * counts_in.astype(np.float64) + (1.0 - _MOMENTUM) * N
    freq = new_counts / new_counts.sum()
    wi = (_REPEAT_THR / freq) ** _GAMMA
    num = float((S * wi).sum() + s_inv)
    den = float((N * wi).sum() + n_inv)
    return np.float32(num / den * _LOSS_WEIGHT)


def _finish_v4(acc, counts_in, numel, dve_mask_edges=None, nchunk=_NCHUNK):
    """acc: [..., P, nchunk*16 + 17] per-core partials from _build_v4."""
    if dve_mask_edges is None:
        dve_mask_edges = _DVE_MASK_EDGES
    a = acc.astype(np.float64)
    a = a.reshape(-1, a.shape[-2], a.shape[-1])
    csums = a[:, :, : nchunk * _NBIN].reshape(-1, _NBIN).sum(axis=0)
    tails = a[:, :, nchunk * _NBIN :].sum(axis=(0, 1))  # [17]
    s_tot = tails[_NBIN]
    C = np.empty(_NBIN)
    T = np.empty(_NBIN)
    for b in range(_NBIN):
        if b < dve_mask_edges:
            C[b] = csums[b]
            T[b] = tails[b]
        else:
            C[b] = (csums[b] + float(numel)) / 2.0
            T[b] = (tails[b] + s_tot) / 2.0
    N = np.empty(_NBIN)
    S = np.empty(_NBIN)
    N[:-1] = C[:-1] - C[1:]
    N[-1] = C[-1]
    S[:-1] = T[:-1] - T[1:]
    S[-1] = T[-1]
    n_inv = numel - C[0]
    s_inv = s_tot - T[0]
    new_counts = _MOMENTUM * counts_in.astype(np.float64) + (1.0 - _MOMENTUM) * N
    freq = new_counts / new_counts.sum()
    wi = (_REPEAT_THR / freq) ** _GAMMA
    num = float((S * wi).sum() + s_inv)
    den = float((N * wi).sum() + n_inv)
    return np.float32(num / den * _LOSS_WEIGHT)


def kernel(outputs, targets, counts):
    outputs = np.asarray(outputs, dtype=np.float32)
    targets = np.asarray(targets, dtype=np.float32)
    counts = np.asarray(counts, dtype=np.float32)
    acc = _run_concat(_shard_inputs(outputs, targets))
    if _VERSION == "v4":
        loss = _finish_v4(acc, counts, outputs.size)
    elif _VERSION == "v3":
        loss = _finish_v3(acc, counts, outputs.size)
    else:
        loss = _finish(acc, counts, outputs.size, counts_mode=_COUNTS_MODE)
    return np.asarray(loss, dtype=np.float32)


def _bench_caller(outputs, targets, repeat):
    """Returns a zero-arg callable timing one sharded call (seconds)."""
    import time as _time

    import jax
    from jax.sharding import NamedSharding, PartitionSpec

    info = _get_exec(repeat=repeat)
    concat_in = _shard_inputs(
        np.asarray(outputs, dtype=np.float32), np.asarray(targets, np.float32)
    )
    sh = NamedSharding(info["mesh"], PartitionSpec("core"))
    dev_args = [
        jax.device_put(concat_in[name], sh) for name in info["in_names"]
    ]
    for a in dev_args:
        a.block_until_ready()

    def one_call():
        zeros = [
            jax.device_put(
                np.zeros((_NCORES * z.shape[0], *z.shape[1:]), z.dtype), sh
            )
            for z in info["zero_outs"]
        ]
        for z in zeros:
            z.block_until_ready()
        t0 = _time.perf_counter()
        outs = info["fn"](*dev_args, *zeros)
        for o in outs:
            o.block_until_ready()
        return _time.perf_counter() - t0

    return one_call


def bench(outputs, targets, r1=2, r2=66, iters=16):
    """Slope-timed per-pass kernel time in ns: the per-call dispatch
    overhead through the axon tunnel (~40-80 ms) swamps a single kernel
    execution, so run the whole pass r1 and r2 times inside one NEFF and
    divide the wall-clock difference by (r2 - r1).  Calls are interleaved
    so slow drift in the tunnel overhead cancels."""
    c1 = _bench_caller(outputs, targets, r1)
    c2 = _bench_caller(outputs, targets, r2)
    c1()
    c2()
    t1s, t2s = [], []
    for _ in range(iters):
        t1s.append(c1())
        t2s.append(c2())
    t1s.sort()
    t2s.sort()
    t1, t2 = t1s[len(t1s) // 4], t2s[len(t2s) // 4]
    per_pass_ns = (t2 - t1) / (r2 - r1) * 1e9
    return per_pass_ns, t1, t2
